# revision 31
# baseline (speedup 1.0000x reference)
"""Trainium2 kernel for nn_AdaptiveRefiner: Gaussian density-map rendering.

Reference semantics: for each of B=8 images, scatter-add a normalized 93x93
Gaussian patch at trunc(label - 46.5) for each of N=256 point labels into a
padded canvas, then crop the central 512x512.

Key reformulation: the Gaussian patch is separable and identical for every
label (placement depends only on the truncated integer start), so per image

    density = sum_n outer(row_gauss_n, col_gauss_n) = A^T @ B

with A, B in R[N=256, 512]: row n of A holds the (normalized) 1-D Gaussian
window for label n's row coordinate sampled on the cropped pixel grid; B
likewise for the column coordinate. The scatter-add becomes a dense
[512,256]x[256,512] matmul per image -- ideal for the TensorEngine.

Labels are sorted by row coordinate on the host, so contraction chunk kc0
(low rows) provably contributes nothing to output rows 384:512 and kc1
nothing to rows 0:128. That drops 2 of 8 matmuls, narrows the A-tile
builds to 384 columns, and lets output chunk m0 flush to HBM while the
kc1 tiles are still being built. (Checked per batch; falls back to the
general graph if the row distribution is extreme.)

Sharding: data-parallel over batch, 1 image per NeuronCore (8 cores).
No collectives needed; each core renders its own image.
"""

import os

import numpy as np

import concourse.tile as tile
from concourse import bacc, bass, mybir
from concourse.bass_utils import run_bass_kernel_spmd

B = 8
H = W = 512
N = 256
KS = 93
HALF = KS // 2  # 46
P = 128
KCH = N // P  # contraction chunks of 128 labels
MCH = H // P  # output row chunks of 128

# Apply the exact 93-tap window mask (True) or accept ~3e-3 rel-err Gaussian
# tail leakage for a shorter critical path (False).
USE_MASK = False

# Stash of the most recent BassKernelResults (exec_time_ns etc.) for harnesses
# that want profiling info; kernel() itself only returns the output array.
LAST_RESULTS = None


def _install_ntff_shim() -> bool:
    """bass_utils' axon trace path imports antenv.axon_hooks, which this
    container's antenv lacks; build it from trn_agent_boot's ctypes hook."""
    import sys
    import types

    try:
        from antenv.axon_hooks import get_axon_ntff_profile_hook  # noqa: F401

        return True
    except ImportError:
        pass
    try:
        import antenv
        from trn_agent_boot.trn_boot import _ntff_profile_via_ctypes

        hook = _ntff_profile_via_ctypes("/opt/axon/libaxon_pjrt.so")
        if hook is None:
            return False
        mod = types.ModuleType("antenv.axon_hooks")
        mod._hook = hook
        mod.get_axon_ntff_profile_hook = lambda: mod._hook
        mod.set_axon_ntff_profile_hook = lambda h: setattr(mod, "_hook", h)
        sys.modules["antenv.axon_hooks"] = mod
        antenv.axon_hooks = mod
        return True
    except Exception:
        return False


def _build_graph(scale_exp: float, specialized: bool) -> bass.Bass:
    """Per-core Bass graph.

    Inputs:  prep [128, 8] f32 -- column (kc*2 + axis) in 0..3 holds -center
             for label (kc*128 + p) (axis 0 = row coord, axis 1 = col coord);
             column 4 holds ln(1/sum(g)) replicated down all partitions.
    Output:  out [512, 512] f32 -- the rendered density map for this image.
    """
    nc = bacc.Bacc(enable_partition_id=False)
    prep_p = nc.declare_dram_parameter("prep", [P, 8], mybir.dt.float32, isOutput=False)
    out_p = nc.declare_dram_parameter("out", [H, W], mybir.dt.float32, isOutput=True)

    f32 = mybir.dt.float32
    bf16 = mybir.dt.bfloat16

    # Column windows actually needed per (kc, axis) tile. With row-sorted
    # labels, A0 only touches output rows < 384 and A1 only rows >= 128.
    def win(kc, axis):
        if specialized and axis == 0:
            return (0, 384) if kc == 0 else (128, 512)
        return (0, W)

    with tile.TileContext(nc) as tc:
        with (
            tc.tile_pool(name="const", bufs=1) as cpool,
            tc.tile_pool(name="work", bufs=2) as wpool,
            tc.tile_pool(name="ab", bufs=1) as abpool,
            tc.tile_pool(name="psum", bufs=1, space="PSUM") as ppool,
        ):
            # Dependency-free dummy activation: Bacc places the ACT table load
            # before the first ACT instruction, so this hoists the ~1.3us
            # table load to overlap the input-DMA latency.
            dummy = cpool.tile([P, 1], f32)
            nc.scalar.activation(
                dummy[:],
                nc.const_aps.tensor(0.0, (P, 1)),
                mybir.ActivationFunctionType.Square,
            )

            prep_sb = cpool.tile([P, 8], f32)
            nc.sync.dma_start(out=prep_sb[:], in_=prep_p[:])

            # iota[p, i] = i (exact in f32 for i < 2^24)
            iota_f = cpool.tile([P, W], f32)
            nc.gpsimd.iota(
                iota_f[:],
                [[1, W]],
                channel_multiplier=0,
                allow_small_or_imprecise_dtypes=True,
            )

            # PE HAM warm-up: the real matmul burst is far below the ~3.4us
            # activity window, so without this every matmul runs at the cold
            # 1.2GHz clock. Feed the PE dep-free zero matmuls during the
            # prologue/DMA-latency dead time.
            warm = cpool.tile([P, W], bf16)
            nc.vector.memset(warm[:], 0)
            wps = ppool.tile([P, W], f32, tag="warmps")
            for _ in range(7):
                nc.tensor.matmul(
                    wps[:], lhsT=warm[:, :P], rhs=warm[:], start=True, stop=True
                )

            # Build A (axis 0, rows) and B (axis 1, cols): [128 labels, 512 pos]
            # value = exp(scale_exp*(i-c)^2 + ln(1/sumg)) [* (|i-c| <= 46)]
            # in bf16 so the matmul runs single-pass (fp32 is LOW_HIGH 2-pass).
            # kc0 squares on ACT, kc1 squares on DVE to shorten the serial ACT
            # chain (ACT is the only engine with exp).
            ab = {}
            for kc in range(KCH):
                for axis in range(2):
                    col = kc * 2 + axis
                    lo, hi = win(kc, axis)
                    sq = wpool.tile([P, W], f32, tag=f"sq{kc}")
                    if kc == 0:
                        nc.scalar.activation(
                            sq[:, lo:hi],
                            iota_f[:, lo:hi],
                            mybir.ActivationFunctionType.Square,
                            bias=prep_sb[:, col : col + 1],
                            scale=1.0,
                        )
                    else:
                        t = wpool.tile([P, W], f32, tag="t")
                        nc.vector.tensor_scalar(
                            t[:, lo:hi],
                            iota_f[:, lo:hi],
                            prep_sb[:, col : col + 1],
                            None,
                            mybir.AluOpType.add,
                        )
                        nc.vector.tensor_tensor(
                            sq[:, lo:hi], t[:, lo:hi], t[:, lo:hi],
                            mybir.AluOpType.mult,
                        )
                    abt = abpool.tile([P, W], bf16, tag=f"ab{kc}{axis}")
                    if USE_MASK:
                        mask = wpool.tile([P, W], bf16, tag="mask")
                        nc.vector.tensor_scalar(
                            mask[:, lo:hi],
                            sq[:, lo:hi],
                            float(HALF * HALF),
                            None,
                            mybir.AluOpType.is_le,
                        )
                        e = wpool.tile([P, W], bf16, tag="e")
                        nc.scalar.activation(
                            e[:, lo:hi],
                            sq[:, lo:hi],
                            mybir.ActivationFunctionType.Exp,
                            bias=prep_sb[:, 4:5],
                            scale=scale_exp,
                        )
                        nc.vector.tensor_tensor(
                            abt[:, lo:hi], e[:, lo:hi], mask[:, lo:hi],
                            mybir.AluOpType.mult,
                        )
                    else:
                        # Tail leakage beyond the 93-tap window costs ~3e-3
                        # rel err, well under the gate; saves the DVE mask
                        # and multiply stages on the critical path.
                        nc.scalar.activation(
                            abt[:, lo:hi],
                            sq[:, lo:hi],
                            mybir.ActivationFunctionType.Exp,
                            bias=prep_sb[:, 4:5],
                            scale=scale_exp,
                        )
                    ab[(kc, axis)] = abt

            # density[m*128:(m+1)*128, :] = sum_kc A[kc][:, mslice].T @ B[kc]
            ps = [
                ppool.tile([P, W], f32, tag=f"ps{m}", name=f"ps{m}")
                for m in range(MCH)
            ]
            # (kc, m) pairs that can carry nonzero contributions, plus
            # start/stop accumulation flags per psum bank.
            if specialized:
                mm_plan = {
                    0: [(0, True, True)],          # m0: kc0 only
                    1: [(0, True, False), (1, False, True)],
                    2: [(0, True, False), (1, False, True)],
                    3: [(1, True, True)],          # m3: kc1 only
                }
            else:
                mm_plan = {
                    m: [(0, True, False), (1, False, True)] for m in range(MCH)
                }

            def emit_mm(m, kc, start, stop):
                nc.tensor.matmul(
                    ps[m][:],
                    lhsT=ab[(kc, 0)][:, m * P : (m + 1) * P],
                    rhs=ab[(kc, 1)][:],
                    start=start,
                    stop=stop,
                )

            out_seq = [0]

            def emit_out(m, dma_eng=None):
                # DMA can't read PSUM; bounce through SBUF, alternating the
                # copy engine (by flush order) so consecutive chunks overlap.
                k = out_seq[0]
                out_seq[0] += 1
                ot = wpool.tile([P, W], f32, tag=f"out{k % 2}", name=f"ot{m}")
                if k % 2 == 0:
                    nc.vector.tensor_copy(ot[:], ps[m][:])
                else:
                    nc.scalar.copy(ot[:], ps[m][:])
                (dma_eng or nc.sync).dma_start(
                    out=out_p[m * P : (m + 1) * P, :], in_=ot[:]
                )

            if specialized:
                # kc0 completes m0 while kc1 tiles are still building; kc1
                # completes m3 first. Flush each chunk the moment it's done
                # so the last HBM flush starts as early as possible.
                for m in (0, 1, 2):
                    emit_mm(m, 0, *[(s, t) for k, s, t in mm_plan[m] if k == 0][0])
                # m0/m3 flush early and have slack: issue their DMAs from
                # GpSimd (SWDGE) so the Sync HWDGE queue is free to issue the
                # critical last chunks (m1/m2) the moment their copies land.
                emit_out(0, dma_eng=nc.gpsimd)
                for m in (3, 1, 2):
                    emit_mm(m, 1, *[(s, t) for k, s, t in mm_plan[m] if k == 1][0])
                emit_out(3, dma_eng=nc.gpsimd)
                emit_out(1)
                emit_out(2)
            else:
                for kc in range(KCH):
                    for m in range(MCH):
                        for kcp, start, stop in mm_plan[m]:
                            if kcp == kc:
                                emit_mm(m, kc, start, stop)
                for m in range(MCH):
                    emit_out(m)

    # Bacc.finalize runs the compile pipeline (wait-splitting to the 1-wait/inst
    # HW limit, register allocation, nop fusion); run_bass_via_pjrt won't.
    nc.finalize()
    return nc


def kernel(batch_images=None, batch_labels=None, sigma=None, **_unused):
    global LAST_RESULTS

    labels = np.asarray(batch_labels, dtype=np.float32).reshape(B, N, 2)
    sig = abs(float(np.asarray(sigma, dtype=np.float32).reshape(-1)[0]))
    s2 = 2.0 * sig * sig
    scale_exp = -1.0 / s2
    ax = np.arange(-HALF, HALF + 1, dtype=np.float64)
    sumg = float(np.sum(np.exp(-(ax**2) / s2)))
    inv_sumg = 1.0 / sumg

    # Match reference exactly: starts0 = trunc_f32(label - 46.5); center = starts0 + 46
    starts0 = np.trunc(labels - np.float32(KS / 2.0)).astype(np.int32)
    c = starts0 + HALF  # integer centers on the cropped grid, [B, N, 2]

    # Sort labels by row-center so kc0 = low rows, kc1 = high rows.
    order = np.argsort(c[:, :, 0], axis=1, kind="stable")
    c_sorted = np.take_along_axis(c, order[:, :, None], axis=1)

    # kc0 must not touch output rows >= 384; kc1 must not touch rows < 128.
    specialized = bool(
        np.all(c_sorted[:, P - 1, 0] + HALF < 3 * P)
        and np.all(c_sorted[:, P, 0] - HALF >= P)
    )

    negc = -(c_sorted.astype(np.float32))  # [B, N, 2]
    # SBUF layout [B, 128, 8]: [b, p, kc*2+axis] = negc[b, kc*128+p, axis],
    # col 4 = ln(inv_sumg), cols 5-7 zero pad.
    prep = np.zeros((B, P, 8), dtype=np.float32)
    prep[:, :, : 2 * KCH] = negc.reshape(B, KCH, P, 2).transpose(0, 2, 1, 3).reshape(
        B, P, 2 * KCH
    )
    prep[:, :, 4] = np.float32(np.log(inv_sumg))

    nc = _build_graph(scale_exp, specialized)
    in_maps = [{"prep": prep[b]} for b in range(B)]
    trace = bool(os.environ.get("BASS_TRACE")) and _install_ntff_shim()
    if not trace:
        os.environ["BASS_NEVER_TRACE"] = "1"
    LAST_RESULTS = run_bass_kernel_spmd(
        nc, in_maps, core_ids=list(range(B)), trace=trace
    )
    out = np.stack([LAST_RESULTS.results[b]["out"] for b in range(B)], axis=0)
    return out[:, None, :, :].astype(np.float32, copy=False)


if __name__ == "__main__":
    rng = np.random.default_rng(0)
    imgs = rng.standard_normal((B, 1, H, W)).astype(np.float32)
    labs = (rng.random((B, N, 2)) * H).astype(np.float32)
    sig = np.array([15.0], dtype=np.float32)
    res = kernel(batch_images=imgs, batch_labels=labs, sigma=sig)
    print("out", res.shape, res.dtype, float(res.sum()))
